# revision 41
# baseline (speedup 1.0000x reference)
"""DualMem retrieval kernel for Trainium2 (8 NeuronCores, Bass/Tile).

Math (per reference):
    sim[b,c,m]  = <img[b], mem[c,m]>
    w           = exp(-beta * (1 - sim))
    adapt[b,c]  = sum_m mem[c,m] * w[b,c,m]
    logits[b,c] = 100 * <img[b], adapt[b,c] / ||adapt[b,c]||>

Key algebraic reduction (avoids materializing adapt [B,C,D]):
    numer[b,c]  = <img[b], adapt[b,c]> = sum_m w[b,c,m] * sim[b,c,m]
    denom[b,c]  = ||adapt[b,c]||^2     = w^T G_c w,  G_c = mem_c @ mem_c^T
    logits      = 100 * numer / sqrt(denom)

Sharding: classes C=1000 split 125 per core across 8 cores.

Precision/layout strategy:
  - mem ships as float8 e3m4 scaled by 32 (1 byte/elem, values in e3m4's
    normal range); host pre-transposes to [d%128, d//128, cm] so every input
    DMA is a plain contiguous copy (no xbar transpose).  img stays bf16; the
    PE accepts mixed e3m4(weights) x bf16(moving) matmuls.  End-to-end rel
    err ~1.05% (gate 2e-2).
  - The x32 scale cancels exactly (numer_psum = 32*numer, denom_psum =
    1024*denom); only the exp gets scale beta/32.
  - Groups g6..g10 additionally ship a float8 e4m3 copy (same x32 scale) so
    their Gram runs as 4 DoubleRow matmuls (256-deep, 0.5 cyc/row) instead
    of 8 plain ones -- Gram precision is insensitive, and those bytes arrive
    when the DMA stream has slack while the PE is still behind.
  - Blocks of [4,4,3,1] groups share PSUM banks:
      su bank [128, gn*128]: group k cols 128k+0:64 = sim, 64:128 = u
      G bank  [128, gn*128]: Gram
      nd bank [16, gn*128] per block: [all numer | all denom]
  - w*sim is computed right after the exp (independent of u) and w*u after
    the u matmul, with separate numer/denom nd matmuls, so only the wu->nd_d
    link sits on each block's critical tail.
  - Block D (last group, 48 rows) packs su|gp|nd into ONE dedicated PSUM
    bank so its sims never wait on the su pool recycling earlier blocks'
    banks; the tile-granular deps inside that bank coincide with the
    block's natural chain.
  - Finals (100/sqrt(d) = exp(-0.5*ln(d)+ln(100)); Ln+Exp share one pinned
    ACT table) run per FIN_PLAN group: blocks A and B individually (their
    nds finish during the PE stream), C+D merged (one Exp, one output DMA)
    to cut the end-of-kernel Act/HWDGE serialization.
  - Fine-grained junk matmuls warm the PE p-state during the DMA startup
    window so real matmuls run at full clock from the start.
"""

import sys

sys.path.insert(0, "/opt/trn_rl_repo")

import ml_dtypes
import numpy as np

B, C, M, D = 64, 1000, 11, 1024
BETA = 5.5
SCALE = 32.0
N_CORES = 8
C_PER = C // N_CORES          # 125 classes per core
CPG = 11                      # classes per group
NG = 12                       # groups per core (132 class slots >= 125)
PG = CPG * M                  # 121 used partitions per group
DCH = D // 128                # 8 d-chunks

IMG_B = 2 * DCH * B           # 1024 bytes/partition of bf16 imgT
M1_B = 2 * 128                # 256
EM_B = 2 * 16                 # 32
GW = [128] * (NG - 1) + [48]  # group col widths (incl zero pad cols)
GB = [DCH * w for w in GW]    # bytes per group per partition
DUAL = {6, 7, 8, 9, 10}       # groups with an extra e4m3 copy for the Gram

# blob layout (per-partition byte cols), ordered to match the DMA chunks:
#  d0: img g0 g1 | d1: g2 g3 m1 em | d2: g4-g7 | d3: g8-g10(+e4m3) | d4: g11
_off = {}
_cur = 0
def _seg(name, nbytes):
    global _cur
    _off[name] = _cur
    _cur += nbytes
_seg("img", IMG_B)
_seg("g0", GB[0]); _seg("g1", GB[1]); _seg("g2", GB[2]); _seg("g3", GB[3])
_seg("m1", M1_B); _seg("em", EM_B); _seg("idx", 2)
for g in range(4, NG):
    _seg(f"g{g}", GB[g])
    if g in DUAL:
        _seg(f"g{g}x", GB[g])
BLOB_B = _cur
# d0: img+g0 (small, earliest PE start) | d1: g1 g2 | d2: g3 consts g4 g5
# | d3: g6 g7 (+e4m3) | d4: g8 g9 (+e4m3) | d5: g10 (+e4m3) | d6: g11
DMA_CUTS = [0, _off["g1"], _off["g3"], _off["g6"], _off["g8"], _off["g10"],
            _off["g11"], _off["g11"] + GB[11]]

BLKS = [(0, 4), (4, 4), (8, 3), (11, 1)]
N_JUNK = 24
N_JUNK_HALF = 1
N_JUNK_Q = 0
SU_BUFS = 2
FIN_PLAN = [[0], [1], [2, 3]]
SCAT_OUT = False  # triggered SWDGE scatter-add: cost-model sem protocol mismatch

_cache = {}


def _build():
    import concourse.mybir as mybir
    import concourse.tile as tile
    from concourse import bacc

    # Pin every activation to the one ACT table that holds BOTH Exp and Ln
    # (indices must be preserved -- empty the other sets instead of dropping
    # them) so the function table is loaded once and never swapped.
    if not getattr(bacc, "_act_tables_pinned", False):
        real = bacc.get_activation_tables

        def pinned(arch):
            return {k: (v if k == "natural_log_exp_and_others" else set())
                    for k, v in real(arch).items()}
        bacc.get_activation_tables = pinned
        bacc._act_tables_pinned = True

    f32 = mybir.dt.float32
    bf16 = mybir.dt.bfloat16
    f8e3 = mybir.dt.float8e3
    f8e4 = mybir.dt.float8e4

    nc = bacc.Bacc("TRN2", target_bir_lowering=False, debug=False,
                   num_devices=N_CORES)

    blob = nc.dram_tensor("blob", [128, BLOB_B], f8e3, kind="ExternalInput")
    out = nc.dram_tensor("out", [16, NG * 64], f32, kind="ExternalOutput")

    with tile.TileContext(nc) as tc:
        with (
            tc.tile_pool(name="const", bufs=1) as const,
            tc.tile_pool(name="sb", bufs=3) as sb,
            tc.tile_pool(name="ps_su", bufs=SU_BUFS, space="PSUM") as ps_su,
            tc.tile_pool(name="ps_g", bufs=2, space="PSUM") as ps_g,
            tc.tile_pool(name="ps_nd", bufs=1, space="PSUM") as ps_nd,
        ):
            # one SBUF tile per DMA chunk so dependency tracking is per-chunk
            cb = [const.tile([128, DMA_CUTS[i + 1] - DMA_CUTS[i]], f8e3,
                             name=f"cb{i}", tag=f"cb{i}")
                  for i in range(len(DMA_CUTS) - 1)]
            for i, t in enumerate(cb):
                nc.sync.dma_start(
                    t[:], blob.ap()[:, DMA_CUTS[i]:DMA_CUTS[i + 1]])

            def view(name, nbytes):
                off = _off[name]
                for i in range(len(cb)):
                    if DMA_CUTS[i] <= off < DMA_CUTS[i + 1]:
                        o = off - DMA_CUTS[i]
                        return cb[i][:, o:o + nbytes]
                raise AssertionError

            imgT = view("img", IMG_B).bitcast(bf16) \
                .rearrange("p (j b) -> p j b", j=DCH)          # [128,8,64]
            m1 = view("m1", M1_B).bitcast(bf16)                # [128,128]
            em_bf = view("em", EM_B).bitcast(bf16)             # [128,16]
            gv_ = [view(f"g{g}", GB[g]).rearrange("p (j w) -> p j w", j=DCH)
                   for g in range(NG)]
            gx_ = {g: view(f"g{g}x", GB[g]).bitcast(f8e4)
                   .rearrange("p (c i w) -> p c i w", c=DCH // 2, i=2)
                   for g in DUAL}

            lg = const.tile([16, NG * 64], f32)
            if SCAT_OUT:
                # last block's output goes through a PREPARED scatter-add
                # fired by trigger_dma: descriptor-gen (HWDGE-equivalent)
                # happens early, so only the transfer + sem sit after the
                # final mul.  The target region is zeroed by an early DMA.
                scat_in = const.tile([128, 256], f32)
                zero_t = const.tile([16, 256], f32)
                idx_v = view("idx", 2).bitcast(mybir.dt.int16)
                nc.vector.memset(scat_in[:], 0)
                nc.vector.memset(zero_t[:], 0)
                nc.sync.dma_start(out.ap()[:, 512:768], zero_t[:])
                dma_sem = nc.alloc_semaphore("swdge_dma")
                nc.gpsimd.dma_scatter_add(
                    out.ap()[0:16, 512:768],
                    scat_in[:].rearrange("p (u e) -> p u e", u=1),
                    idx_v[0:16],
                    16, 16, 256, elem_step=NG * 64,
                    prepare_only=True, sem=dma_sem)
            bias_exp = const.tile([128, 1], f32)
            bias_eps = const.tile([16, 1], f32)
            bias_ln100 = const.tile([16, 1], f32)
            junk_w = const.tile([128, 16], bf16)
            junk_x = const.tile([128, 128], bf16)
            nc.vector.memset(junk_w[:], 0)
            nc.vector.memset(junk_x[:], 0)
            nc.vector.memset(bias_exp[:], -BETA)
            nc.vector.memset(bias_eps[:], 1e-30)
            nc.vector.memset(bias_ln100[:], float(np.log(100.0)))

            # nd tiles: [all numer | all denom] per block; blocks C and D
            # share one bank via disjoint cols (all nd writes are emitted
            # before any nd read, so tile-level dep tracking adds no false
            # serialization), freeing a bank for a third su buffer
            # block D (1 group, 48 rows) packs su|gp|nd into ONE bank so
            # its sims never wait on the su pool recycling earlier blocks'
            # banks; the tile-granular deps inside the bank coincide with
            # the block's natural chain
            nd_a = ps_nd.tile([16, 512], f32, name="nd_a")
            nd_b = ps_nd.tile([16, 512], f32, name="nd_b")
            # C and D interleave one bank as [numC | numD | denC | denD] so
            # the merged finals are ONE Ln + ONE Exp + ONE mul over
            # contiguous ranges (all writes precede all reads, so the
            # tile-granular deps add no false serialization)
            nd_cd = ps_nd.tile([16, 512], f32, name="nd_cd")
            gnD = BLKS[3][1]
            bigD = ps_nd.tile([128, 512 * gnD], f32, name="bigD")
            nC = BLKS[2][1] * 64
            nd_nd = [
                (nd_a[:, 0:BLKS[0][1] * 64], nd_a[:, 256:256 + BLKS[0][1] * 64]),
                (nd_b[:, 0:BLKS[1][1] * 64], nd_b[:, 256:256 + BLKS[1][1] * 64]),
                (nd_cd[:, 0:nC], nd_cd[:, 256:256 + nC]),
                (nd_cd[:, nC:256], nd_cd[:, 256 + nC:512]),
            ]

            # PE warm-up: the HAM clock gate (and the cost model) halve the
            # PE clock until ~3us of sustained activity.  Fine-grained junk
            # matmuls (no DMA dependency) run during the DMA startup window;
            # they scribble on nd_a, rewritten (start=True) later.
            for _ in range(N_JUNK):
                nc.tensor.matmul(nd_a[:, 0:128], junk_w[:], junk_x[:],
                                 start=True, stop=True,
                                 skip_group_check=True)
            for _ in range(N_JUNK_HALF):
                nc.tensor.matmul(nd_a[:, 0:64], junk_w[:], junk_x[:, 0:64],
                                 start=True, stop=True,
                                 skip_group_check=True)
            for _ in range(N_JUNK_Q):
                nc.tensor.matmul(nd_a[:, 0:32], junk_w[:], junk_x[:, 0:32],
                                 start=True, stop=True,
                                 skip_group_check=True)

            def emit_sims(nb, g0, gn):
                if nb == len(BLKS) - 1:
                    su = bigD[:, 0:128 * gn]
                    gp = bigD[:, 128 * gn:256 * gn]
                else:
                    su = ps_su.tile([128, gn * 128], f32, tag="su",
                                    name=f"su{nb}")
                    gp = ps_g.tile([128, gn * 128], f32, tag="gp",
                                   name=f"gp{nb}")
                for k in range(gn):
                    g = g0 + k
                    gw = GW[g]
                    gvv = 48 if g == NG - 1 else PG  # valid Gram columns
                    for i in range(DCH):
                        nc.tensor.matmul(su[0:gw, k * 128:k * 128 + 64],
                                         gv_[g][:, i, 0:gw], imgT[:, i, :],
                                         start=(i == 0), stop=(i == DCH - 1),
                                         skip_group_check=True)
                    if g in DUAL:
                        # e4m3 DoubleRow Gram: 4 x 256-deep at 0.5 cyc/row
                        for c in range(DCH // 2):
                            gvi = gw if c == 0 else gvv
                            nc.tensor.matmul(
                                gp[0:gw, k * 128:k * 128 + gvi],
                                gx_[g][:, c, :, 0:gw],
                                gx_[g][:, c, :, 0:gvi],
                                start=(c == 0), stop=(c == DCH // 2 - 1),
                                perf_mode=mybir.MatmulPerfMode.DoubleRow,
                                skip_group_check=True)
                    else:
                        for i in range(DCH):
                            # i==0 writes all gw cols so the masked read
                            # later never sees uninitialized PSUM; pad cols
                            # keep the i==0 partial and are masked out
                            gvi = gw if i == 0 else gvv
                            nc.tensor.matmul(
                                gp[0:gw, k * 128:k * 128 + gvi],
                                gv_[g][:, i, 0:gw], gv_[g][:, i, 0:gvi],
                                start=(i == 0), stop=(i == DCH - 1),
                                skip_group_check=True)
                return su, gp

            def emit_down(nb, gn, su, gp):
                gw = 128 if BLKS[nb][0] + gn < NG else (48 if gn == 1 else 128)
                su = su[0:gw]
                gp = gp[0:gw]
                ndn, ndd = nd_nd[nb]
                # w = exp((beta/SCALE)*sim_psum - beta) for the whole block
                su4 = su.rearrange("p (k t b) -> p k t b", k=gn, t=2)
                w4 = sb.tile([128, gn * 64], bf16, tag="w4",
                             name=f"w4_{nb}")[0:gw]
                w4r = w4.rearrange("p (k b) -> p k b", k=gn)
                nc.scalar.activation(w4r, su4[:, :, 0, :],
                                     mybir.ActivationFunctionType.Exp,
                                     bias=bias_exp[0:gw],
                                     scale=BETA / SCALE)

                # masked Gram -> SBUF (kills cross-class + pad entries)
                gm4 = sb.tile([128, gn * 128], bf16, tag="gm4",
                              name=f"gm4_{nb}")[0:gw]
                if gn == 1:
                    nc.vector.tensor_mul(gm4[:, 0:gw], gp[:, 0:gw],
                                         m1[0:gw, 0:gw])
                else:
                    gp4 = gp.rearrange("p (k j) -> p k j", k=gn)
                    m1b = m1.rearrange("p (u j) -> p u j", u=1) \
                        .to_broadcast((gw, gn, 128))
                    nc.vector.tensor_mul(
                        gm4.rearrange("p (k j) -> p k j", k=gn), gp4, m1b)

                # w*sim immediately (independent of u) -> numer matmul early
                ws = sb.tile([128, gn * 64], bf16, tag="ws",
                             name=f"ws_{nb}")[0:gw]
                ws3 = ws.rearrange("p (k b) -> p k b", k=gn)
                nc.vector.tensor_mul(ws3, su4[:, :, 0, :], w4r)
                nc.tensor.matmul(ndn, em_bf[0:gw], ws,
                                 start=True, stop=True, skip_group_check=True)

                # u_k = G_k^T @ w_k, placed next to sim_k in the same bank
                for k in range(gn):
                    nc.tensor.matmul(su[:, k * 128 + 64:(k + 1) * 128],
                                     gm4[:, k * 128:k * 128 + gw],
                                     w4[:, k * 64:(k + 1) * 64],
                                     start=True, stop=True,
                                     skip_group_check=True)

                # w*u -> denom matmul
                wu = sb.tile([128, gn * 64], bf16, tag="wu",
                             name=f"wu_{nb}")[0:gw]
                wu3 = wu.rearrange("p (k b) -> p k b", k=gn)
                nc.vector.tensor_mul(wu3, su4[:, :, 1, :], w4r)
                nc.tensor.matmul(ndd, em_bf[0:gw], wu,
                                 start=True, stop=True, skip_group_check=True)

            def emit_finals(blocks):
                # 100/sqrt(denom) = exp(-0.5*ln(denom) + ln(100)); Ln and Exp
                # live in the same ACT table, so no table swap ever.  Merged
                # blocks share one Exp and one output DMA to cut the
                # end-of-kernel Act/HWDGE serialization.
                tag = "f" + "".join(str(nb) for nb in blocks)
                tot = sum(BLKS[nb][1] for nb in blocks) * 64
                if len(blocks) == 1:
                    num_ap, den_ap = nd_nd[blocks[0]]
                else:
                    num_ap, den_ap = nd_cd[:, 0:256], nd_cd[:, 256:512]
                if len(blocks) > 1 and tot <= 256 * gnD:
                    # Ln scratch in bigD's spare PSUM half: Act PSUM access
                    # beats SBUF on the write and the Exp's read
                    s_h = bigD[0:16, 256 * gnD:256 * gnD + tot]
                else:
                    s_h = sb.tile([16, tot], f32, tag=f"s{tag}",
                                  name=f"s_{tag}")
                r_h = sb.tile([16, tot], f32, tag=f"r{tag}",
                              name=f"r_{tag}")
                nc.scalar.activation(s_h[:], den_ap,
                                     mybir.ActivationFunctionType.Ln,
                                     bias=bias_eps[:], scale=1.0)
                nc.scalar.activation(r_h[:], s_h[:],
                                     mybir.ActivationFunctionType.Exp,
                                     bias=bias_ln100[:], scale=-0.5)
                return num_ap, r_h

            def emit_fin_mul(blocks, num_ap, r_h):
                # the DVE mul + out DMA have slack; emitting them after every
                # gm/ws/wu keeps the DVE queue from head-blocking the masked
                # Gram of the late blocks behind final muls
                lo = BLKS[blocks[0]][0] * 64
                hi = (BLKS[blocks[-1]][0] + BLKS[blocks[-1]][1]) * 64
                nc.vector.tensor_mul(lg[:, lo:hi], num_ap, r_h[:])
                nc.sync.dma_start(out.ap()[:, lo:hi], lg[:, lo:hi])

            # Emission order: block sims ahead of the previous block's
            # downstream (the PE queue never head-blocks on exp/mask); each
            # block's finals are emitted where the in-order Act queue has an
            # idle window AFTER that block's nd is complete and BEFORE the
            # next exp becomes ready, so no exp is ever head-blocked.
            suA, gpA = emit_sims(0, *BLKS[0])
            suB, gpB = emit_sims(1, *BLKS[1])
            emit_down(0, BLKS[0][1], suA, gpA)
            suC, gpC = emit_sims(2, *BLKS[2])
            emit_down(1, BLKS[1][1], suB, gpB)
            f0 = emit_finals(FIN_PLAN[0])
            suD, gpD = emit_sims(3, *BLKS[3])
            emit_down(2, BLKS[2][1], suC, gpC)
            emit_down(3, BLKS[3][1], suD, gpD)
            f1 = emit_finals(FIN_PLAN[1])
            emit_fin_mul(FIN_PLAN[0], *f0)
            emit_fin_mul(FIN_PLAN[1], *f1)
            f2 = emit_finals(FIN_PLAN[2])
            emit_fin_mul(FIN_PLAN[2], *f2)

    nc.compile()
    return nc


def _get_nc():
    if "nc" not in _cache:
        _cache["nc"] = _build()
    return _cache["nc"]


def _prep_inputs(img_features, memorized_image_feat):
    """Host-side formatting: dtype casts, x32 scale, transpose, group blob."""
    bf = ml_dtypes.bfloat16
    f8 = ml_dtypes.float8_e3m4
    f8x = ml_dtypes.float8_e4m3
    img_b = np.ascontiguousarray(img_features.astype(bf))       # [64, 1024]
    mem_s = memorized_image_feat * SCALE                        # [1000,11,1024]
    mem8 = mem_s.astype(f8)

    # imgT bytes: it[p, j, b] = img[b, j*128+p]
    it = img_b.T.reshape(DCH, 128, B).transpose(1, 0, 2)        # [128, 8, 64]
    it_bytes = np.ascontiguousarray(it).view(np.uint8).reshape(128, IMG_B)

    m1 = np.zeros((128, 128), np.float32)
    for c in range(CPG):
        m1[c * M:(c + 1) * M, c * M:(c + 1) * M] = 1.0
    m1_bytes = m1.astype(bf).view(np.uint8).reshape(128, M1_B)
    em = np.zeros((128, 16), np.float32)
    for c in range(CPG):
        em[c * M:(c + 1) * M, c] = 1.0
    em_bytes = em.astype(bf).view(np.uint8).reshape(128, EM_B)

    def tr_bytes(rows, gw):                  # rows [gw, D] -> [128, 8*gw] u8
        mt = rows.T.reshape(DCH, 128, gw).transpose(1, 0, 2)
        return np.ascontiguousarray(mt).view(np.uint8).reshape(128, DCH * gw)

    in_maps = []
    for k in range(N_CORES):
        lo = k * C_PER * M
        sl = mem8.reshape(C * M, D)[lo:lo + C_PER * M]
        slx = mem_s.reshape(C * M, D)[lo:lo + C_PER * M]
        blob = np.zeros((128, BLOB_B), np.uint8)
        blob[:, _off["img"]:_off["img"] + IMG_B] = it_bytes
        blob[:, _off["m1"]:_off["m1"] + M1_B] = m1_bytes
        blob[:, _off["em"]:_off["em"] + EM_B] = em_bytes
        for g in range(NG):
            r0, gw = g * PG, GW[g]
            n = min(PG, C_PER * M - r0)
            rows = np.zeros((gw, D), f8)
            rows[:n] = sl[r0:r0 + n]
            blob[:, _off[f"g{g}"]:_off[f"g{g}"] + GB[g]] = tr_bytes(rows, gw)
            if g in DUAL:
                rowsx = np.zeros((gw, D), f8x)
                rowsx[:n] = slx[r0:r0 + n].astype(f8x)
                blob[:, _off[f"g{g}x"]:_off[f"g{g}x"] + GB[g]] = \
                    tr_bytes(rowsx, gw)
        in_maps.append({"blob": blob.view(f8)})
    return in_maps


def _gather(results):
    logits = np.empty((B, C), np.float32)
    for k in range(N_CORES):
        o = results[k]["out"].reshape(16, NG, 64)[:CPG]         # [11, 12, 64]
        o = o.transpose(1, 0, 2).reshape(NG * CPG, 64)[:C_PER]  # [125, 64]
        logits[:, k * C_PER:(k + 1) * C_PER] = o.T
    return logits


def kernel(img_features, memorized_image_feat):
    from concourse.bass_utils import run_bass_kernel_spmd

    nc = _get_nc()
    in_maps = _prep_inputs(img_features, memorized_image_feat)
    res = run_bass_kernel_spmd(nc, in_maps, core_ids=list(range(N_CORES)))
    return _gather(res.results)
